# revision 6
# baseline (speedup 1.0000x reference)
"""BERT self-attention (S=1024, B=4, H=1024, 16 heads x 64 dim) on 8 trn2 cores.

Sharding: batch*heads split across 8 cores (8 heads each, b = core//2,
head block = core%2). Each core computes, for its 8 heads:
  QT = (Wq_c @ X_b^T) [j, s]   (j = head-major qkv dim, 512 per core)
  KT likewise, V = (X_b @ Wv_c^T) [t, j] (natural orientation)
  ST = K Q^T scaled+mask -> exp (no max-subtract; scores are O(5) so exp
       is safely in fp32 range), giving E [t, s] per head
  CT_aug = [V_h | 1]^T E  -> rows 0..63 unnormalized ctx^T, row 64 = softmax
       denominator (ones-column trick), then divide on-chip.
Host does layout-only work: slicing, transposes, and final reassembly.
"""

import numpy as np

import concourse.bacc as bacc
import concourse.mybir as mybir
import concourse.tile as tile
from concourse.bass_utils import run_bass_kernel_spmd

F32 = mybir.dt.float32
F32R = mybir.dt.float32r
BF16 = mybir.dt.bfloat16
I32 = mybir.dt.int32

S = 1024          # sequence length
B = 4             # batch
H = 1024          # hidden
HEADS = 16
D = 64            # head dim
N_CORES = 8
HPC = 8           # heads per core
JPC = HPC * D     # qkv dim per core = 512
KT_TILES = H // 128   # 8 contraction tiles
TB = S // 128         # 8 t-blocks
SB = S // 512         # 2 s-blocks (matmul free dim 512)

_CACHE: dict = {}


def _build():
    nc = bacc.Bacc("TRN2", target_bir_lowering=False, debug=False,
                   num_devices=N_CORES)

    xt_d = nc.dram_tensor("xt", [H, S], F32R, kind="ExternalInput").ap()
    wqt_d = nc.dram_tensor("wqt", [H, JPC], F32R, kind="ExternalInput").ap()
    wkt_d = nc.dram_tensor("wkt", [H, JPC], F32R, kind="ExternalInput").ap()
    wvt_d = nc.dram_tensor("wvt", [H, JPC], F32R, kind="ExternalInput").ap()
    bq_d = nc.dram_tensor("bq", [JPC], F32, kind="ExternalInput").ap()
    bk_d = nc.dram_tensor("bk", [JPC], F32, kind="ExternalInput").ap()
    bv_d = nc.dram_tensor("bv", [JPC], F32R, kind="ExternalInput").ap()
    maskt_d = nc.dram_tensor("maskt", [S, HPC], F32, kind="ExternalInput").ap()
    niter_d = nc.dram_tensor("niter", [1, 1], I32, kind="ExternalInput").ap()
    ones_d = nc.dram_tensor("ones", [1, 128], F32R, kind="ExternalInput").ap()
    out_d = nc.dram_tensor("out", [HPC, D, S], F32, kind="ExternalOutput").ap()

    with tile.TileContext(nc) as tc:
        with (
            tc.tile_pool(name="ctrl", bufs=1) as ctrl_pool,
            tc.tile_pool(name="xt", bufs=KT_TILES) as xt_pool,
            tc.tile_pool(name="wqk", bufs=4) as wqk_pool,
            tc.tile_pool(name="wv", bufs=1) as wv_pool,
            tc.tile_pool(name="qk", bufs=8) as qk_pool,
            tc.tile_pool(name="v", bufs=TB) as v_pool,
            tc.tile_pool(name="e", bufs=24) as e_pool,
            tc.tile_pool(name="small", bufs=4) as small_pool,
            tc.tile_pool(name="norm", bufs=3) as norm_pool,
            tc.tile_pool(name="cout", bufs=3) as cout_pool,
            tc.tile_pool(name="proj_ps", bufs=2, space="PSUM") as proj_ps,
            tc.tile_pool(name="score_ps", bufs=2, space="PSUM") as score_ps,
            tc.tile_pool(name="ct_ps", bufs=2, space="PSUM") as ct_ps,
        ):
            nit = ctrl_pool.tile([1, 1], I32)
            nc.sync.dma_start(nit[:], niter_d[:])
            n_reps = nc.values_load(nit[0:1, 0:1], min_val=1, max_val=1 << 20,
                                    skip_runtime_bounds_check=True)

            with tc.For_i(0, n_reps, 1,
                          hint_engines=(mybir.EngineType.PE,)):
                # ---- constants / small inputs ----
                bq_sb = small_pool.tile([128, JPC // 128], F32, tag="bias")
                nc.sync.dma_start(bq_sb[:], bq_d.rearrange("(o p) -> p o", p=128))
                bk_sb = small_pool.tile([128, JPC // 128], F32, tag="bias")
                nc.sync.dma_start(bk_sb[:], bk_d.rearrange("(o p) -> p o", p=128))
                bv_row = small_pool.tile([1, JPC], F32R, tag="bvrow")
                nc.sync.dma_start(bv_row[:], bv_d[None, :])
                mask_sb = small_pool.tile([128, TB, HPC], F32, tag="mask")
                nc.sync.dma_start(
                    mask_sb[:], maskt_d.rearrange("(tb p) h -> p tb h", p=128))
                ones_sb = small_pool.tile([1, 128], F32R, tag="ones")
                nc.sync.dma_start(ones_sb[:], ones_d[:])

                # ---- X^T tiles ----
                xt_t = []
                xt_r = xt_d.rearrange("(o p) s -> o p s", p=128)
                for kt in range(KT_TILES):
                    t = xt_pool.tile([128, S], F32R, tag="xt")
                    nc.sync.dma_start(t[:], xt_r[kt])
                    xt_t.append(t)

                def load_w_jt(dram, jt):
                    """[128, kt=8, 128] tile: column slice jt of W^T."""
                    t = wqk_pool.tile([128, KT_TILES, 128], F32R, tag="wqk")
                    nc.sync.dma_start(
                        t[:], dram.rearrange("(o p) j -> p o j", p=128)
                        [:, :, jt * 128:(jt + 1) * 128])
                    return t

                q_tiles: list = [None] * 4
                k_tiles: list = [None] * 4

                def project_qk(w_jt, bias_sb, dst_tiles, jt):
                    """QT/KT j-tile jt: [128 j, 1024 s] = W^T.T @ X^T."""
                    dst = qk_pool.tile([128, S], F32R, tag="qk")
                    for sb in range(SB):
                        ps = proj_ps.tile([128, 512], F32, tag="pps")
                        for kt in range(KT_TILES):
                            nc.tensor.matmul(
                                ps[:],
                                lhsT=w_jt[:, kt, :],
                                rhs=xt_t[kt][:, sb * 512:(sb + 1) * 512]
                                ,
                                start=(kt == 0), stop=(kt == KT_TILES - 1))
                        nc.vector.tensor_scalar_add(
                            dst[:, sb * 512:(sb + 1) * 512], ps[:],
                            bias_sb[:, jt:jt + 1])
                    dst_tiles[jt] = dst

                # ---- V projection (natural [t, j] orientation) ----
                v_tiles = []

                def project_v(wv_sb):
                    for tb in range(TB):
                        ps = proj_ps.tile([128, 512], F32, tag="pps")
                        for kt in range(KT_TILES):
                            nc.tensor.matmul(
                                ps[:],
                                lhsT=xt_t[kt][:, tb * 128:(tb + 1) * 128]
                                ,
                                rhs=wv_sb[:, kt, :],
                                start=(kt == 0), stop=False)
                        # += ones^T (x) bv   (broadcast bias over t rows)
                        nc.tensor.matmul(
                            ps[:], lhsT=ones_sb[:],
                            rhs=bv_row[:],
                            start=False, stop=True)
                        vt = v_pool.tile([128, HPC * (D + 1)], BF16, tag="v")
                        v3 = vt[:].rearrange("p (h d) -> p h d", d=D + 1)
                        nc.vector.tensor_copy(
                            out=v3[:, :, 0:D],
                            in_=ps[:].rearrange("p (h d) -> p h d", d=D))
                        nc.vector.memset(v3[:, :, D:D + 1], 1.0)
                        v_tiles.append(vt)

                def scores_exp(h, e_dst):
                    """ST=[t,s] per t-block -> exp -> E tiles (bf16)."""
                    jt, off = h // 2, (h % 2) * 64
                    for tb in range(TB):
                        sp = score_ps.tile([128, S], F32, tag="sps")
                        for sb in range(SB):
                            nc.tensor.matmul(
                                sp[:, sb * 512:(sb + 1) * 512],
                                lhsT=k_tiles[jt][off:off + 64,
                                                 tb * 128:(tb + 1) * 128]
                                ,
                                rhs=q_tiles[jt][off:off + 64,
                                                sb * 512:(sb + 1) * 512]
                                ,
                                start=True, stop=True)
                        e = e_pool.tile([128, S], BF16, tag="e")
                        nc.scalar.activation(
                            e[:], sp[:], mybir.ActivationFunctionType.Exp,
                            bias=mask_sb[:, tb, h:h + 1], scale=0.125)
                        e_dst[tb] = e

                def av(h, e_src):
                    """CT_aug[65, s] = [V_h|1]^T @ E; normalize; DMA out."""
                    for sb in range(SB):
                        ct = ct_ps.tile([D + 1, 512], F32, tag="ct")
                        for tb in range(TB):
                            nc.tensor.matmul(
                                ct[:],
                                lhsT=v_tiles[tb][:, h * (D + 1):
                                                 (h + 1) * (D + 1)],
                                rhs=e_src[tb][:, sb * 512:(sb + 1) * 512],
                                start=(tb == 0), stop=(tb == TB - 1))
                        rc = norm_pool.tile([1, 512], F32, tag="recip")
                        nc.vector.reciprocal(rc[:], ct[D:D + 1, :])
                        bc = norm_pool.tile([D, 512], F32, tag="bcast")
                        nc.gpsimd.partition_broadcast(bc[:], rc[:])
                        co = cout_pool.tile([D, 512], F32, tag="cout")
                        nc.vector.tensor_tensor(co[:], ct[0:D, :], bc[:],
                                                mybir.AluOpType.mult)
                        nc.sync.dma_start(
                            out_d[h, :, sb * 512:(sb + 1) * 512], co[:])

                # ---- emission order: pipeline projections with attention ----
                project_qk(load_w_jt(wqt_d, 0), bq_sb, q_tiles, 0)
                project_qk(load_w_jt(wkt_d, 0), bk_sb, k_tiles, 0)

                e_tiles: dict = {h: [None] * TB for h in range(HPC)}
                scores_exp(0, e_tiles[0])
                scores_exp(1, e_tiles[1])

                wv_sb = wv_pool.tile([128, KT_TILES, JPC], F32R, tag="wv")
                nc.sync.dma_start(
                    wv_sb[:], wvt_d.rearrange("(o p) j -> p o j", p=128))
                project_v(wv_sb)

                av(0, e_tiles[0])
                av(1, e_tiles[1])

                for jt in range(1, 4):
                    project_qk(load_w_jt(wqt_d, jt), bq_sb, q_tiles, jt)
                    project_qk(load_w_jt(wkt_d, jt), bk_sb, k_tiles, jt)
                    scores_exp(2 * jt, e_tiles[2 * jt])
                    scores_exp(2 * jt + 1, e_tiles[2 * jt + 1])
                    av(2 * jt, e_tiles[2 * jt])
                    av(2 * jt + 1, e_tiles[2 * jt + 1])

    nc.compile()
    return nc


def _get_nc():
    if "nc" not in _CACHE:
        _CACHE["nc"] = _build()
    return _CACHE["nc"]


def _shard_inputs(hidden_states, attention_mask, Wq, bq, Wk, bk, Wv, bv,
                  n_reps=1):
    in_maps = []
    for c in range(N_CORES):
        b = c // 2
        js = slice((c % 2) * JPC, (c % 2) * JPC + JPC)
        ns = slice(c * HPC, (c + 1) * HPC)
        in_maps.append({
            "xt": np.ascontiguousarray(hidden_states[:, b, :].T),
            "wqt": np.ascontiguousarray(Wq[js, :].T),
            "wkt": np.ascontiguousarray(Wk[js, :].T),
            "wvt": np.ascontiguousarray(Wv[js, :].T),
            "bq": np.ascontiguousarray(bq[js]),
            "bk": np.ascontiguousarray(bk[js]),
            "bv": np.ascontiguousarray(bv[js]),
            "maskt": np.ascontiguousarray(attention_mask[ns, 0, :].T),
            "niter": np.array([[n_reps]], dtype=np.int32),
            "ones": np.ones((1, 128), dtype=np.float32),
        })
    return in_maps


def _gather_outputs(results):
    out = np.empty((S, B, H), dtype=np.float32)
    for c in range(N_CORES):
        ct = results[c]["out"]          # (HPC, D, S)
        b = c // 2
        for hl in range(HPC):
            hg = (c % 2) * HPC + hl
            out[:, b, hg * D:(hg + 1) * D] = ct[hl].T
    return out


def run(n_reps, **inputs):
    nc = _get_nc()
    in_maps = _shard_inputs(n_reps=n_reps, **{
        k: np.asarray(v) for k, v in inputs.items()})
    res = run_bass_kernel_spmd(nc, in_maps, list(range(N_CORES)))
    return _gather_outputs(res.results)


def kernel(**inputs):
    return run(1, **inputs)


# revision 17
# speedup vs baseline: 19.8513x; 19.8513x over previous
"""BERT self-attention (S=1024, B=4, H=1024, 16 heads x 64 dim) on 8 trn2 cores.

Sharding: batch*heads split across 8 cores (8 heads each, b = core//2,
head block = core%2). Each core computes, for its 8 heads:
  QT = (Wq_c @ X_b^T) [j, s]   (j = head-major qkv dim, 512 per core)
  KT likewise, V = (X_b @ Wv_c^T) [t, j] (natural orientation)
  ST = K Q^T scaled+mask -> exp (no max-subtract; scores are O(5) so exp
       is safely in fp32 range), giving E [t, s] per head
  CT_aug = [V_h | 1]^T E  -> rows 0..63 unnormalized ctx^T, row 64 = softmax
       denominator (ones-column trick), then divide on-chip.
Host does layout-only work: slicing, transposes, and final reassembly.
"""

import numpy as np

import concourse.bacc as bacc
import concourse.mybir as mybir
import concourse.tile as tile
from concourse.bass_utils import run_bass_kernel_spmd

F32 = mybir.dt.float32
F32R = mybir.dt.float32r
BF16 = mybir.dt.bfloat16
I32 = mybir.dt.int32

S = 1024          # sequence length
B = 4             # batch
H = 1024          # hidden
HEADS = 16
D = 64            # head dim
N_CORES = 8
HPC = 8           # heads per core
JPC = HPC * D     # qkv dim per core = 512
KT_TILES = H // 128   # 8 contraction tiles
TB = S // 128         # 8 t-blocks
SB = S // 512         # 2 s-blocks (matmul free dim 512)

_CACHE: dict = {}


def _build():
    nc = bacc.Bacc("TRN2", target_bir_lowering=False, debug=False,
                   num_devices=N_CORES)

    xt_d = nc.dram_tensor("xt", [H, S], BF16, kind="ExternalInput").ap()
    wqt_d = nc.dram_tensor("wqt", [H, JPC], BF16, kind="ExternalInput").ap()
    wkt_d = nc.dram_tensor("wkt", [H, JPC], BF16, kind="ExternalInput").ap()
    wvt_d = nc.dram_tensor("wvt", [H, JPC], BF16, kind="ExternalInput").ap()
    bq_d = nc.dram_tensor("bq", [128, JPC // 128], F32, kind="ExternalInput").ap()
    bk_d = nc.dram_tensor("bk", [128, JPC // 128], F32, kind="ExternalInput").ap()
    bv_d = nc.dram_tensor("bv", [JPC], BF16, kind="ExternalInput").ap()
    maskt_d = nc.dram_tensor("maskt", [128, TB, HPC], F32, kind="ExternalInput").ap()
    niter_d = nc.dram_tensor("niter", [1, 1], I32, kind="ExternalInput").ap()
    ones_d = nc.dram_tensor("ones", [1, 128], BF16, kind="ExternalInput").ap()
    out_d = nc.dram_tensor("out", [HPC, S, D], F32, kind="ExternalOutput").ap()

    with tile.TileContext(nc) as tc:
        with (
            tc.tile_pool(name="ctrl", bufs=1) as ctrl_pool,
            tc.tile_pool(name="xt", bufs=KT_TILES) as xt_pool,
            tc.tile_pool(name="wqk", bufs=4) as wqk_pool,
            tc.tile_pool(name="wv", bufs=1) as wv_pool,
            tc.tile_pool(name="qk", bufs=8) as qk_pool,
            tc.tile_pool(name="v", bufs=TB) as v_pool,
            tc.tile_pool(name="e", bufs=24) as e_pool,
            tc.tile_pool(name="small", bufs=4) as small_pool,
            tc.tile_pool(name="norm", bufs=3) as norm_pool,
            tc.tile_pool(name="cout", bufs=3) as cout_pool,
            tc.tile_pool(name="proj_ps", bufs=2, space="PSUM") as proj_ps,
            tc.tile_pool(name="score_ps", bufs=2, space="PSUM") as score_ps,
            tc.tile_pool(name="ct_ps", bufs=2, space="PSUM") as ct_ps,
        ):
            nit = ctrl_pool.tile([1, 1], I32)
            nc.sync.dma_start(nit[:], niter_d[:])
            n_reps = nc.values_load(nit[0:1, 0:1], min_val=1, max_val=1 << 20,
                                    skip_runtime_bounds_check=True)

            with tc.For_i(0, n_reps, 1,
                          hint_engines=(mybir.EngineType.PE,)):
                # ---- constants / small inputs ----
                bq_sb = small_pool.tile([128, JPC // 128], F32, tag="bias")
                nc.sync.dma_start(bq_sb[:], bq_d[:])
                bk_sb = small_pool.tile([128, JPC // 128], F32, tag="bias")
                nc.sync.dma_start(bk_sb[:], bk_d[:])
                bv_row = small_pool.tile([1, JPC], BF16, tag="bvrow")
                nc.sync.dma_start(bv_row[:], bv_d[None, :])
                mask_sb = small_pool.tile([128, TB, HPC], F32, tag="mask")
                nc.sync.dma_start(mask_sb[:], maskt_d[:])
                ones_sb = small_pool.tile([1, 128], BF16, tag="ones")
                nc.sync.dma_start(ones_sb[:], ones_d[:])

                # ---- X^T tiles ----
                xt_t = []
                xt_r = xt_d.rearrange("(o p) s -> o p s", p=128)
                for kt in range(KT_TILES):
                    t = xt_pool.tile([128, S], BF16, tag="xt")
                    nc.sync.dma_start(t[:], xt_r[kt])
                    xt_t.append(t)

                def load_w_jt(dram, jt):
                    """[128, kt=8, 128] tile: column slice jt of W^T."""
                    t = wqk_pool.tile([128, KT_TILES, 128], BF16, tag="wqk")
                    nc.sync.dma_start(
                        t[:], dram.rearrange("(o p) j -> p o j", p=128)
                        [:, :, jt * 128:(jt + 1) * 128])
                    return t

                q_tiles: list = [None] * 4
                k_tiles: list = [None] * 4

                def project_qk(w_jt, bias_sb, dst_tiles, jt):
                    """QT/KT j-tile jt: [128 j, 1024 s] = W^T.T @ X^T."""
                    dst = qk_pool.tile([128, S], BF16, tag="qk")
                    for sb in range(SB):
                        ps = proj_ps.tile([128, 512], F32, tag="pps")
                        for kt in range(KT_TILES):
                            nc.tensor.matmul(
                                ps[:],
                                lhsT=w_jt[:, kt, :],
                                rhs=xt_t[kt][:, sb * 512:(sb + 1) * 512]
                                ,
                                start=(kt == 0), stop=(kt == KT_TILES - 1))
                        nc.vector.tensor_scalar_add(
                            dst[:, sb * 512:(sb + 1) * 512], ps[:],
                            bias_sb[:, jt:jt + 1])
                    dst_tiles[jt] = dst

                # ---- V projection (natural [t, j] orientation) ----
                v_tiles = []

                def project_v(wv_sb):
                    for tb in range(TB):
                        ps = proj_ps.tile([128, 512], F32, tag="pps")
                        for kt in range(KT_TILES):
                            nc.tensor.matmul(
                                ps[:],
                                lhsT=xt_t[kt][:, tb * 128:(tb + 1) * 128]
                                ,
                                rhs=wv_sb[:, kt, :],
                                start=(kt == 0), stop=False)
                        # += ones^T (x) bv   (broadcast bias over t rows)
                        nc.tensor.matmul(
                            ps[:], lhsT=ones_sb[:],
                            rhs=bv_row[:],
                            start=False, stop=True)
                        vt = v_pool.tile([128, HPC * (D + 1)], BF16, tag="v")
                        v3 = vt[:].rearrange("p (h d) -> p h d", d=D + 1)
                        nc.vector.tensor_copy(
                            out=v3[:, :, 0:D],
                            in_=ps[:].rearrange("p (h d) -> p h d", d=D))
                        nc.vector.memset(v3[:, :, D:D + 1], 1.0)
                        v_tiles.append(vt)

                def scores_exp(h, e_dst):
                    """ST=[t,s] per t-block -> exp -> E tiles (bf16)."""
                    jt, off = h // 2, (h % 2) * 64
                    for tb in range(TB):
                        sp = score_ps.tile([128, S], F32, tag="sps")
                        for sb in range(SB):
                            nc.tensor.matmul(
                                sp[:, sb * 512:(sb + 1) * 512],
                                lhsT=k_tiles[jt][off:off + 64,
                                                 tb * 128:(tb + 1) * 128],
                                rhs=q_tiles[jt][off:off + 64,
                                                sb * 512:(sb + 1) * 512],
                                start=True, stop=True)
                        e = e_pool.tile([128, S], BF16, tag="e")
                        nc.scalar.activation(
                            e[:], sp[:], mybir.ActivationFunctionType.Exp,
                            bias=mask_sb[:, tb, h:h + 1], scale=0.125)
                        e_dst[tb] = e

                def av(h, e_src):
                    """C_aug[s,65] = E_slice^T @ [V_h|1]; col 64 = softmax
                    denominator (per-partition) -> recip + scalar-mul."""
                    co = cout_pool.tile([128, S // 128, D], F32, tag="cout")
                    for sblk in range(S // 128):
                        ct = ct_ps.tile([128, D + 1], F32, tag="ct")
                        for tb in range(TB):
                            nc.tensor.matmul(
                                ct[:],
                                lhsT=e_src[tb][:, sblk * 128:(sblk + 1) * 128],
                                rhs=v_tiles[tb][:, h * (D + 1):
                                                (h + 1) * (D + 1)],
                                start=(tb == 0), stop=(tb == TB - 1))
                        rc = norm_pool.tile([128, 1], F32, tag="recip")
                        nc.vector.reciprocal(rc[:], ct[:, D:D + 1])
                        nc.vector.tensor_scalar_mul(
                            co[:, sblk, :], ct[:, 0:D], rc[:])
                    nc.sync.dma_start(
                        out_d[h].rearrange("(sblk p) d -> p sblk d", p=128),
                        co[:])

                # ---- emission order: pipeline projections with attention ----
                project_qk(load_w_jt(wqt_d, 0), bq_sb, q_tiles, 0)
                project_qk(load_w_jt(wkt_d, 0), bk_sb, k_tiles, 0)

                e_tiles: dict = {h: [None] * TB for h in range(HPC)}
                scores_exp(0, e_tiles[0])
                scores_exp(1, e_tiles[1])

                wv_sb = wv_pool.tile([128, KT_TILES, JPC], BF16, tag="wv")
                nc.sync.dma_start(
                    wv_sb[:], wvt_d.rearrange("(o p) j -> p o j", p=128))
                project_v(wv_sb)

                av(0, e_tiles[0])
                av(1, e_tiles[1])

                for jt in range(1, 4):
                    project_qk(load_w_jt(wqt_d, jt), bq_sb, q_tiles, jt)
                    project_qk(load_w_jt(wkt_d, jt), bk_sb, k_tiles, jt)
                    scores_exp(2 * jt, e_tiles[2 * jt])
                    scores_exp(2 * jt + 1, e_tiles[2 * jt + 1])
                    av(2 * jt, e_tiles[2 * jt])
                    av(2 * jt + 1, e_tiles[2 * jt + 1])

    nc.compile()
    return nc


def _get_nc():
    if "nc" not in _CACHE:
        _CACHE["nc"] = _build()
    return _CACHE["nc"]


def _shard_inputs(hidden_states, attention_mask, Wq, bq, Wk, bk, Wv, bv,
                  n_reps=1):
    import ml_dtypes
    bf16 = ml_dtypes.bfloat16
    in_maps = []
    for c in range(N_CORES):
        b = c // 2
        js = slice((c % 2) * JPC, (c % 2) * JPC + JPC)
        ns = slice(c * HPC, (c + 1) * HPC)
        in_maps.append({
            "xt": np.ascontiguousarray(hidden_states[:, b, :].T).astype(bf16),
            "wqt": np.ascontiguousarray(Wq[js, :].T).astype(bf16),
            "wkt": np.ascontiguousarray(Wk[js, :].T).astype(bf16),
            "wvt": np.ascontiguousarray(Wv[js, :].T).astype(bf16),
            "bq": np.ascontiguousarray(bq[js].reshape(4, 128).T),
            "bk": np.ascontiguousarray(bk[js].reshape(4, 128).T),
            "bv": np.ascontiguousarray(bv[js]).astype(bf16),
            "maskt": np.ascontiguousarray(
                attention_mask[ns, 0, :].T.reshape(8, 128, 8)
                .transpose(1, 0, 2)),
            "niter": np.array([[n_reps]], dtype=np.int32),
            "ones": np.ones((1, 128), dtype=bf16),
        })
    return in_maps


def _gather_outputs(results):
    out = np.empty((S, B, H), dtype=np.float32)
    for c in range(N_CORES):
        ct = results[c]["out"]          # (HPC, S, D)
        b = c // 2
        for hl in range(HPC):
            hg = (c % 2) * HPC + hl
            out[:, b, hg * D:(hg + 1) * D] = ct[hl]
    return out


def run(n_reps, **inputs):
    nc = _get_nc()
    in_maps = _shard_inputs(n_reps=n_reps, **{
        k: np.asarray(v) for k, v in inputs.items()})
    try:
        res = run_bass_kernel_spmd(nc, in_maps, list(range(N_CORES)))
    except Exception:
        # transient axon/PJRT hiccups occasionally surface as INTERNAL errors;
        # a single retry on the same compiled program is usually enough
        res = run_bass_kernel_spmd(nc, in_maps, list(range(N_CORES)))
    return _gather_outputs(res.results)


def kernel(**inputs):
    return run(1, **inputs)
